# revision 44
# baseline (speedup 1.0000x reference)
"""Weighted-MAE loss (nn_MAELoss) on 8 Trainium2 NeuronCores.

reference:  w = bucket-weights(y_true) via thresholds log1p(5/25/50),
            loss = sum(w * |y_true - y_pred|) / sum(w)

Strategy: data-parallel over the batch dim (8 shards of 8 batches). Inputs
are cast to bf16 on the host (rel err ~2.7e-4, far inside the 2e-2 gate),
halving HBM traffic: 7.86 MB/core -> ~21.8us DMA floor at 360 GB/s.

Per-core dataflow (all stock ops; every engine stays under the DMA floor):
  DMA   : yt/yp stream into full resident SBUF buffers in column chunks
          on the SP queue (which carries nothing else, so it never stalls).
  DVE   : tensor_scalar is_ge builds the three bf16 threshold masks into a
          packed [m1|m2|m3|ones] stationary layout (4x perf mode, bucket
          counts fall out of accum_out); most spans' d = yt - yp run here
          too (2x perf mode).
  Pool  : a few mid-stream spans take BOTH their sub and abs on GPSIMD
          ("vertical" ownership) so a slow Pool op never head-blocks the
          in-order DVE/ACT queues.
  ACT   : absd = Abs(d) for the DVE-owned spans.
  PE    : per 40-col microtile, matmul with stationary = [m1|m2|m3|ones]
          (121 cols) and moving = absd (40 cols), all accumulated into one
          PSUM bank. psum[40k+i, j] += sum_p m_k[p,i]*absd[p,j] and row 120
          accumulates plain column sums of absd. The host reads the three
          40-wide diagonal bands (S_k = sum(m_k * |d|)) and row 120 (S0) --
          no elementwise product pass and no reduction pass ever run.
The host combines counts and S0..S3 in float64 and divides.
"""

import os
import sys

import numpy as np

try:
    import concourse  # noqa: F401
except ImportError:  # pragma: no cover
    for _p in ("/root/.axon_site/_ro/trn_rl_repo", "/opt/trn_rl_repo"):
        if os.path.isdir(_p) and _p not in sys.path:
            sys.path.append(_p)

from contextlib import ExitStack

import concourse.bacc as bacc
import concourse.tile as tile
from concourse import mybir
from concourse.bass_utils import run_bass_kernel_spmd

# ----------------------------------------------------------------- problem
N_CORES = 8
B, C, T, H, W = 64, 1, 15, 128, 128
SHARD_B = B // N_CORES
P = 128
F = SHARD_B * C * T * H * W // P  # 15360
N_TOTAL = B * C * T * H * W      # 15728640

TW = 40                   # microtile width (3*TW + 1 = 121 <= 128 stationary)
NT = F // TW              # 384 microtiles
SW = 3 * TW + 1           # stationary width incl. ones column
NP = SW                   # psum partition rows
assert NT * TW == F

THR1 = float(np.float32(np.log1p(5.0)))
THR2 = float(np.float32(np.log1p(25.0)))
THR3 = float(np.float32(np.log1p(50.0)))
W_BASE = 0.2
DW1, DW2, DW3 = 29.8, 2470.0, 17500.0

# DMA chunks (tiles): a chunk pair needs >= ~22 tiles for its transfer
# time to cover the 2x625ns HWDGE fixed cost, so only the first chunk
# (compute warm-up) and the last (drain) are small
CHUNKS_T = [8, 24, 24, 32, 40, 48, 48, 48, 48, 32, 16, 12, 4]
assert sum(CHUNKS_T) == NT
# mask-op groups == chunks (fine-grained waits at every boundary)
MGROUPS = [(i,) for i in range(len(CHUNKS_T))]
NG = len(MGROUPS)
# sub/abs work spans (tiles): chunk-aligned splits of <= 20 tiles
SPANS_T = [8, 12, 12, 12, 12, 16, 16, 20, 20, 16, 16, 16, 16, 16, 16,
           16, 16, 16, 16, 16, 16, 16, 16, 16, 12, 4]
assert sum(SPANS_T) == NT
NS = len(SPANS_T)
# spans whose sub runs on GPSIMD (alternating mid spans, none near the
# tail); their abs and matmuls are emitted LAG spans late so slow GPSIMD
# work never head-blocks the in-order ACT/PE queues
POOL_SPANS = {5, 7, 9, 11, 13, 15, 17, 19}
LAG = 1
# tail spans where |d| comes from a DVE relu pair (relu(d), relu(-d))
# with doubled matmuls, keeping ACT's backlog off the drain chain
RELU_SPANS = {NS - 2, NS - 1}

ND = 3 * NG           # acc slots: (c1, c2, c3) per mask group
NOUT = ND + TW        # + staged psum cols

_STATE: dict = {}


def _build():
    if "nc" in _STATE:
        return _STATE["nc"]
    f32 = mybir.dt.float32
    bf16 = mybir.dt.bfloat16
    A = mybir.AluOpType
    nc = bacc.Bacc("TRN2", target_bir_lowering=False, debug=False,
                   enable_asserts=False)
    yt_d = nc.dram_tensor("y_true", [P, NT, TW], bf16, kind="ExternalInput").ap()
    yp_d = nc.dram_tensor("y_pred", [P, NT, TW], bf16, kind="ExternalInput").ap()
    acc_d = nc.dram_tensor("partials", [P, NOUT], f32, kind="ExternalOutput").ap()

    chunk_ends = np.cumsum(CHUNKS_T).tolist()
    chunk_starts = [0] + chunk_ends[:-1]
    span_ends = np.cumsum(SPANS_T).tolist()
    span_starts = [0] + span_ends[:-1]

    with tile.TileContext(nc) as tc, ExitStack() as ctx:
        big_pool = ctx.enter_context(tc.tile_pool(name="big", bufs=1))
        acc_pool = ctx.enter_context(tc.tile_pool(name="acc", bufs=1))
        ps_pool = ctx.enter_context(tc.psum_pool(name="ps", bufs=1))

        yt = big_pool.tile([P, NT, TW], bf16, tag="yt")
        yp = big_pool.tile([P, NT, TW], bf16, tag="yp")
        masks = big_pool.tile([P, NT, SW], bf16, tag="masks")
        # d holds yt-yp per span, then |d| (or relu(d)) after the in-place
        # second pass (a single full-size tensor: no rotating-buffer WAR
        # stalls, minimal SBUF)
        dfull = big_pool.tile([P, NT, TW], bf16, tag="d")
        # relu(-d) halves for the RELU_SPANS tail tiles
        rneg = big_pool.tile([P, 16, TW], bf16, tag="rneg")
        acc = acc_pool.tile([P, NOUT], f32, tag="acc")
        psum = ps_pool.tile([NP, TW], f32, tag="ps")

        # ones column of the stationary (psum row 120 = column sums of absd);
        # on Pool, which is otherwise idle until mid-stream
        nc.gpsimd.memset(masks[:, :, 3 * TW:SW], 1.0)

        # --- DMA order: the tail chunks' yt is pulled EARLY so their mask
        # work is done long before the stream ends; the last-landing data
        # (tail yp) then needs only sub+relu+matmul in the drain ------------
        NCH = len(CHUNKS_T)
        TAIL_YT = [NCH - 3, NCH - 2, NCH - 1]
        dma_order = [("yt", 0), ("yp", 0)]
        dma_order += [("yt", c) for c in TAIL_YT]
        for ci in range(1, NCH):
            dma_order.append(("yt", ci) if ci not in TAIL_YT else None)
            dma_order.append(("yp", ci))
        dma_order = [x for x in dma_order if x]
        spans_of_chunk = {}
        for si in range(NS):
            ci = next(i for i in range(NCH)
                      if chunk_starts[i] <= span_starts[si] < chunk_ends[i])
            assert span_ends[si] <= chunk_ends[ci], "span straddles chunk"
            spans_of_chunk.setdefault(ci, []).append(si)
        events = []
        for kind, ci in dma_order:
            events.append((kind, ci))

        pend_abs = []      # Pool spans whose abs emission is lagged
        abs_done = set()
        masks_done = set()
        mm_queue = []      # spans pending PE emission
        span_chunk = {si: ci for ci, sis in spans_of_chunk.items()
                      for si in sis}

        relu_off = {}
        for _off, _si in enumerate(sorted(RELU_SPANS)):
            relu_off[_si] = sum(SPANS_T[s] for s in sorted(RELU_SPANS)[:_off])

        def emit_abs(si):
            s0, s1 = span_starts[si], span_ends[si]
            if si in RELU_SPANS:
                # relu(-d) into scratch first (reads d), then relu(d)
                # in-place -- all on DVE at 4x, no ACT hop in the drain
                r0 = relu_off[si]
                r1 = r0 + (s1 - s0)
                nc.vector.tensor_scalar(rneg[:, r0:r1, :], dfull[:, s0:s1, :],
                                        0.0, -1.0, A.min, A.mult)
                nc.vector.tensor_scalar(dfull[:, s0:s1, :], dfull[:, s0:s1, :],
                                        0.0, 1.0, A.max, A.mult)
            else:
                nc.scalar.activation(dfull[:, s0:s1, :], dfull[:, s0:s1, :],
                                     mybir.ActivationFunctionType.Abs)
            abs_done.add(si)

        def emit_matmuls(si):
            # psum accumulation is order-independent; only the start
            # (span 0, tile 0, emitted first) and stop (span NS-1, tile
            # NT-1, emitted last) flags are order-sensitive
            s0, s1 = span_starts[si], span_ends[si]
            for tt in range(s0, s1):
                last = tt == NT - 1
                nc.tensor.matmul(
                    psum[:, :],
                    masks[:, tt, :],          # [P, SW] stationary
                    dfull[:, tt, :],          # [P, TW] moving (|d| or relu)
                    start=tt == 0, stop=last and si not in RELU_SPANS)
                if si in RELU_SPANS:
                    rt = relu_off[si] + (tt - s0)
                    nc.tensor.matmul(
                        psum[:, :],
                        masks[:, tt, :],
                        rneg[:, rt, :],       # relu(-d) half
                        start=False, stop=last)

        def drain_mm(final=False):
            # emit any span whose masks and |d| already exist (Tile
            # discovers deps in emission order, and the in-order PE queue
            # would head-block on a not-yet-ready span's matmuls); span 0
            # must go first and span NS-1 last
            for qsi in sorted(mm_queue):
                if span_chunk[qsi] not in masks_done or qsi not in abs_done:
                    continue
                if qsi == NS - 1 and not (final and len(mm_queue) == 1):
                    continue
                if 0 in mm_queue and qsi != 0:
                    continue
                mm_queue.remove(qsi)
                emit_matmuls(qsi)

        for kind, ci in events:
            c0, c1 = chunk_starts[ci], chunk_ends[ci]
            if kind == "yt":
                nc.sync.dma_start(yt[:, c0:c1, :], yt_d[:, c0:c1, :])
                for k, thr in enumerate((THR1, THR2, THR3)):
                    # with accum_out, op1 is the reduction op: accum=sum(mask)
                    nc.vector.tensor_scalar(
                        masks[:, c0:c1, k * TW:(k + 1) * TW],
                        yt[:, c0:c1, :], thr, 0.0,
                        A.is_ge, A.add,
                        accum_out=acc[:, 3 * ci + k:3 * ci + k + 1])
                masks_done.add(ci)
                drain_mm()
                continue
            nc.sync.dma_start(yp[:, c0:c1, :], yp_d[:, c0:c1, :])
            for si in spans_of_chunk[ci]:
                s0, s1 = span_starts[si], span_ends[si]
                yt_s, yp_s = yt[:, s0:s1, :], yp[:, s0:s1, :]
                d_s = dfull[:, s0:s1, :]
                on_pool = si in POOL_SPANS
                if on_pool:
                    nc.gpsimd.tensor_sub(d_s, yt_s, yp_s)
                else:
                    nc.vector.tensor_sub(d_s, yt_s, yp_s)
                mm_queue.append(si)
                if on_pool:
                    pend_abs.append(si)
                else:
                    emit_abs(si)
                while pend_abs and si - pend_abs[0] >= LAG:
                    emit_abs(pend_abs.pop(0))
                drain_mm()
        for qsi in pend_abs:
            emit_abs(qsi)
        drain_mm(final=True)
        assert not mm_queue

        # --- drain: stage psum into acc's tail region, one output DMA ------
        nc.vector.tensor_copy(acc[0:NP, ND:NOUT], psum[:, :])
        nc.sync.dma_start(acc_d[:], acc[:])

    nc.compile()
    _STATE["nc"] = nc
    return nc


def _run_device(y_pred: np.ndarray, y_true: np.ndarray, **kw):
    import ml_dtypes
    nc = _build()
    y_pred = np.asarray(y_pred, dtype=np.float32).reshape(B, -1)
    y_true = np.asarray(y_true, dtype=np.float32).reshape(B, -1)
    in_maps = []
    for c in range(N_CORES):
        sl = slice(c * SHARD_B, (c + 1) * SHARD_B)
        in_maps.append({
            "y_true": np.ascontiguousarray(y_true[sl]).astype(
                ml_dtypes.bfloat16).reshape(P, NT, TW),
            "y_pred": np.ascontiguousarray(y_pred[sl]).astype(
                ml_dtypes.bfloat16).reshape(P, NT, TW),
        })
    return run_bass_kernel_spmd(nc, in_maps, list(range(N_CORES)), **kw)


def _finalize(results) -> np.ndarray:
    c = np.zeros(3, dtype=np.float64)
    s_band = np.zeros(3, dtype=np.float64)
    s0 = 0.0
    for ci in range(N_CORES):
        part = results[ci]["partials"].astype(np.float64)
        sl = part[:, :ND].sum(axis=0).reshape(NG, 3)
        c += sl.sum(axis=0)
        prods = part[0:NP, ND:NOUT]  # staged psum [121, TW]
        for k in range(3):
            s_band[k] += np.trace(prods[k * TW:(k + 1) * TW, :])
        s0 += prods[3 * TW, :].sum()
    num = W_BASE * s0 + DW1 * s_band[0] + DW2 * s_band[1] + DW3 * s_band[2]
    den = W_BASE * N_TOTAL + DW1 * c[0] + DW2 * c[1] + DW3 * c[2]
    return np.array(num / den, dtype=np.float32)


def kernel(y_pred: np.ndarray, y_true: np.ndarray) -> np.ndarray:
    try:
        res = _run_device(y_pred, y_true)
    except Exception:
        import time as _time
        _time.sleep(2.0)
        res = _run_device(y_pred, y_true)
    return _finalize(res.results)


# revision 47
# speedup vs baseline: 1.0348x; 1.0348x over previous
"""Weighted-MAE loss (nn_MAELoss) on 8 Trainium2 NeuronCores.

reference:  w = bucket-weights(y_true) via thresholds log1p(5/25/50),
            loss = sum(w * |y_true - y_pred|) / sum(w)

Strategy: data-parallel over the batch dim (8 shards of 8 batches). Inputs
are cast to bf16 on the host (rel err ~2.7e-4, far inside the 2e-2 gate),
halving HBM traffic: 7.86 MB/core -> ~21.8us DMA floor at 360 GB/s.

Per-core dataflow (all stock ops; every engine stays under the DMA floor):
  DMA   : yt/yp stream into full resident SBUF buffers in column chunks
          on the SP queue (which carries nothing else, so it never stalls).
  DVE   : tensor_scalar is_ge builds the three bf16 threshold masks into a
          packed [m1|m2|m3|ones] stationary layout (4x perf mode, bucket
          counts fall out of accum_out); most spans' d = yt - yp run here
          too (2x perf mode).
  Pool  : a few mid-stream spans take BOTH their sub and abs on GPSIMD
          ("vertical" ownership) so a slow Pool op never head-blocks the
          in-order DVE/ACT queues.
  ACT   : absd = Abs(d) for the DVE-owned spans.
  PE    : per 40-col microtile, matmul with stationary = [m1|m2|m3|ones]
          (121 cols) and moving = absd (40 cols), all accumulated into one
          PSUM bank. psum[40k+i, j] += sum_p m_k[p,i]*absd[p,j] and row 120
          accumulates plain column sums of absd. The host reads the three
          40-wide diagonal bands (S_k = sum(m_k * |d|)) and row 120 (S0) --
          no elementwise product pass and no reduction pass ever run.
The host combines counts and S0..S3 in float64 and divides.
"""

import os
import sys

import numpy as np

try:
    import concourse  # noqa: F401
except ImportError:  # pragma: no cover
    for _p in ("/root/.axon_site/_ro/trn_rl_repo", "/opt/trn_rl_repo"):
        if os.path.isdir(_p) and _p not in sys.path:
            sys.path.append(_p)

from contextlib import ExitStack

import concourse.bacc as bacc
import concourse.tile as tile
from concourse import mybir
from concourse.bass_utils import run_bass_kernel_spmd

# ----------------------------------------------------------------- problem
N_CORES = 8
B, C, T, H, W = 64, 1, 15, 128, 128
SHARD_B = B // N_CORES
P = 128
F = SHARD_B * C * T * H * W // P  # 15360
N_TOTAL = B * C * T * H * W      # 15728640

TW = 40                   # microtile width (3*TW + 1 = 121 <= 128 stationary)
NT = F // TW              # 384 microtiles
SW = 3 * TW + 1           # stationary width incl. ones column
NP = SW                   # psum partition rows
assert NT * TW == F

THR1 = float(np.float32(np.log1p(5.0)))
THR2 = float(np.float32(np.log1p(25.0)))
THR3 = float(np.float32(np.log1p(50.0)))
W_BASE = 0.2
DW1, DW2, DW3 = 29.8, 2470.0, 17500.0

# DMA chunks (tiles): a chunk pair needs >= ~22 tiles for its transfer
# time to cover the 2x625ns HWDGE fixed cost, so only the first chunk
# (compute warm-up) and the last (drain) are small
CHUNKS_T = [8, 24, 24, 32, 40, 48, 48, 48, 48, 32, 16, 12, 4]
assert sum(CHUNKS_T) == NT
# mask-op groups == chunks, except the three tail chunks whose yt is
# prefetched as one merged early DMA (one mask group, HWDGE-clean size)
MGROUPS = [(i,) for i in range(len(CHUNKS_T) - 3)] + \
    [(len(CHUNKS_T) - 3, len(CHUNKS_T) - 2, len(CHUNKS_T) - 1)]
NG = len(MGROUPS)
# sub/abs work spans (tiles): chunk-aligned splits of <= 20 tiles
SPANS_T = [8, 12, 12, 12, 12, 16, 16, 20, 20, 16, 16, 16, 16, 16, 16,
           16, 16, 16, 16, 16, 16, 16, 16, 16, 12, 4]
assert sum(SPANS_T) == NT
NS = len(SPANS_T)
# spans whose sub runs on GPSIMD (alternating mid spans, none near the
# tail); their abs and matmuls are emitted LAG spans late so slow GPSIMD
# work never head-blocks the in-order ACT/PE queues
POOL_SPANS = {5, 7, 9, 11, 13, 15, 17, 19}
LAG = 1
# tail spans where |d| comes from a DVE relu pair (relu(d), relu(-d))
# with doubled matmuls, keeping ACT's backlog off the drain chain
RELU_SPANS = {NS - 2, NS - 1}

ND = 3 * NG           # acc slots: (c1, c2, c3) per mask group
NOUT = ND + TW        # + staged psum cols

_STATE: dict = {}


def _build():
    if "nc" in _STATE:
        return _STATE["nc"]
    f32 = mybir.dt.float32
    bf16 = mybir.dt.bfloat16
    A = mybir.AluOpType
    nc = bacc.Bacc("TRN2", target_bir_lowering=False, debug=False,
                   enable_asserts=False)
    yt_d = nc.dram_tensor("y_true", [P, NT, TW], bf16, kind="ExternalInput").ap()
    yp_d = nc.dram_tensor("y_pred", [P, NT, TW], bf16, kind="ExternalInput").ap()
    acc_d = nc.dram_tensor("partials", [P, NOUT], f32, kind="ExternalOutput").ap()

    chunk_ends = np.cumsum(CHUNKS_T).tolist()
    chunk_starts = [0] + chunk_ends[:-1]
    span_ends = np.cumsum(SPANS_T).tolist()
    span_starts = [0] + span_ends[:-1]

    with tile.TileContext(nc) as tc, ExitStack() as ctx:
        big_pool = ctx.enter_context(tc.tile_pool(name="big", bufs=1))
        acc_pool = ctx.enter_context(tc.tile_pool(name="acc", bufs=1))
        ps_pool = ctx.enter_context(tc.psum_pool(name="ps", bufs=1))

        yt = big_pool.tile([P, NT, TW], bf16, tag="yt")
        yp = big_pool.tile([P, NT, TW], bf16, tag="yp")
        masks = big_pool.tile([P, NT, SW], bf16, tag="masks")
        # d holds yt-yp per span, then |d| (or relu(d)) after the in-place
        # second pass (a single full-size tensor: no rotating-buffer WAR
        # stalls, minimal SBUF)
        dfull = big_pool.tile([P, NT, TW], bf16, tag="d")
        # relu(-d) halves for the RELU_SPANS tail tiles
        rneg = big_pool.tile([P, 16, TW], bf16, tag="rneg")
        acc = acc_pool.tile([P, NOUT], f32, tag="acc")
        psum = ps_pool.tile([NP, TW], f32, tag="ps")

        # ones column of the stationary (psum row 120 = column sums of absd);
        # on Pool, which is otherwise idle until mid-stream
        nc.gpsimd.memset(masks[:, :, 3 * TW:SW], 1.0)

        # --- DMA order: the tail chunks' yt is pulled EARLY so their mask
        # work is done long before the stream ends; the last-landing data
        # (tail yp) then needs only sub+relu+matmul in the drain ------------
        NCH = len(CHUNKS_T)
        TAIL_YT = [NCH - 3, NCH - 2, NCH - 1]
        dma_order = [("yt", 0), ("yp", 0), ("yt_tail", None)]
        for ci in range(1, NCH):
            if ci not in TAIL_YT:
                dma_order.append(("yt", ci))
            dma_order.append(("yp", ci))
        spans_of_chunk = {}
        for si in range(NS):
            ci = next(i for i in range(NCH)
                      if chunk_starts[i] <= span_starts[si] < chunk_ends[i])
            assert span_ends[si] <= chunk_ends[ci], "span straddles chunk"
            spans_of_chunk.setdefault(ci, []).append(si)
        events = []
        for kind, ci in dma_order:
            events.append((kind, ci))

        pend_abs = []      # Pool spans whose abs emission is lagged
        abs_done = set()
        masks_done = set()
        mm_queue = []      # spans pending PE emission
        span_chunk = {si: ci for ci, sis in spans_of_chunk.items()
                      for si in sis}

        relu_off = {}
        for _off, _si in enumerate(sorted(RELU_SPANS)):
            relu_off[_si] = sum(SPANS_T[s] for s in sorted(RELU_SPANS)[:_off])

        def emit_abs(si):
            s0, s1 = span_starts[si], span_ends[si]
            if si in RELU_SPANS:
                # relu(-d) into scratch first (reads d), then relu(d)
                # in-place -- all on DVE at 4x, no ACT hop in the drain
                r0 = relu_off[si]
                r1 = r0 + (s1 - s0)
                nc.vector.tensor_scalar(rneg[:, r0:r1, :], dfull[:, s0:s1, :],
                                        0.0, -1.0, A.min, A.mult)
                nc.vector.tensor_scalar(dfull[:, s0:s1, :], dfull[:, s0:s1, :],
                                        0.0, 1.0, A.max, A.mult)
            else:
                nc.scalar.activation(dfull[:, s0:s1, :], dfull[:, s0:s1, :],
                                     mybir.ActivationFunctionType.Abs)
            abs_done.add(si)

        def emit_matmuls(si):
            # psum accumulation is order-independent; only the start
            # (span 0, tile 0, emitted first) and stop (span NS-1, tile
            # NT-1, emitted last) flags are order-sensitive
            s0, s1 = span_starts[si], span_ends[si]
            for tt in range(s0, s1):
                last = tt == NT - 1
                nc.tensor.matmul(
                    psum[:, :],
                    masks[:, tt, :],          # [P, SW] stationary
                    dfull[:, tt, :],          # [P, TW] moving (|d| or relu)
                    start=tt == 0, stop=last and si not in RELU_SPANS)
                if si in RELU_SPANS:
                    rt = relu_off[si] + (tt - s0)
                    nc.tensor.matmul(
                        psum[:, :],
                        masks[:, tt, :],
                        rneg[:, rt, :],       # relu(-d) half
                        start=False, stop=last)

        def drain_mm(final=False):
            # emit any span whose masks and |d| already exist (Tile
            # discovers deps in emission order, and the in-order PE queue
            # would head-block on a not-yet-ready span's matmuls); span 0
            # must go first and span NS-1 last
            for qsi in sorted(mm_queue):
                if span_chunk[qsi] not in masks_done or qsi not in abs_done:
                    continue
                if qsi == NS - 1 and not (final and len(mm_queue) == 1):
                    continue
                if 0 in mm_queue and qsi != 0:
                    continue
                mm_queue.remove(qsi)
                emit_matmuls(qsi)

        def emit_masks(g0, g1, slot):
            for k, thr in enumerate((THR1, THR2, THR3)):
                # with accum_out, op1 is the reduction op: accum=sum(mask)
                nc.vector.tensor_scalar(
                    masks[:, g0:g1, k * TW:(k + 1) * TW],
                    yt[:, g0:g1, :], thr, 0.0,
                    A.is_ge, A.add,
                    accum_out=acc[:, 3 * slot + k:3 * slot + k + 1])

        for kind, ci in events:
            if kind == "yt_tail":
                g0, g1 = chunk_starts[TAIL_YT[0]], chunk_ends[TAIL_YT[-1]]
                nc.sync.dma_start(yt[:, g0:g1, :], yt_d[:, g0:g1, :])
                emit_masks(g0, g1, NG - 1)
                masks_done.update(TAIL_YT)
                drain_mm()
                continue
            c0, c1 = chunk_starts[ci], chunk_ends[ci]
            if kind == "yt":
                nc.sync.dma_start(yt[:, c0:c1, :], yt_d[:, c0:c1, :])
                emit_masks(c0, c1, ci)
                masks_done.add(ci)
                drain_mm()
                continue
            nc.sync.dma_start(yp[:, c0:c1, :], yp_d[:, c0:c1, :])
            for si in spans_of_chunk[ci]:
                s0, s1 = span_starts[si], span_ends[si]
                yt_s, yp_s = yt[:, s0:s1, :], yp[:, s0:s1, :]
                d_s = dfull[:, s0:s1, :]
                on_pool = si in POOL_SPANS
                if on_pool:
                    nc.gpsimd.tensor_sub(d_s, yt_s, yp_s)
                else:
                    nc.vector.tensor_sub(d_s, yt_s, yp_s)
                mm_queue.append(si)
                if on_pool:
                    pend_abs.append(si)
                else:
                    emit_abs(si)
                while pend_abs and si - pend_abs[0] >= LAG:
                    emit_abs(pend_abs.pop(0))
                drain_mm()
        for qsi in pend_abs:
            emit_abs(qsi)
        drain_mm(final=True)
        assert not mm_queue

        # --- drain: stage psum into acc's tail region, one output DMA ------
        nc.vector.tensor_copy(acc[0:NP, ND:NOUT], psum[:, :])
        nc.sync.dma_start(acc_d[:], acc[:])

    nc.compile()
    _STATE["nc"] = nc
    return nc


def _run_device(y_pred: np.ndarray, y_true: np.ndarray, **kw):
    import ml_dtypes
    nc = _build()
    y_pred = np.asarray(y_pred, dtype=np.float32).reshape(B, -1)
    y_true = np.asarray(y_true, dtype=np.float32).reshape(B, -1)
    in_maps = []
    for c in range(N_CORES):
        sl = slice(c * SHARD_B, (c + 1) * SHARD_B)
        in_maps.append({
            "y_true": np.ascontiguousarray(y_true[sl]).astype(
                ml_dtypes.bfloat16).reshape(P, NT, TW),
            "y_pred": np.ascontiguousarray(y_pred[sl]).astype(
                ml_dtypes.bfloat16).reshape(P, NT, TW),
        })
    return run_bass_kernel_spmd(nc, in_maps, list(range(N_CORES)), **kw)


def _finalize(results) -> np.ndarray:
    c = np.zeros(3, dtype=np.float64)
    s_band = np.zeros(3, dtype=np.float64)
    s0 = 0.0
    for ci in range(N_CORES):
        part = results[ci]["partials"].astype(np.float64)
        sl = part[:, :ND].sum(axis=0).reshape(NG, 3)
        c += sl.sum(axis=0)
        prods = part[0:NP, ND:NOUT]  # staged psum [121, TW]
        for k in range(3):
            s_band[k] += np.trace(prods[k * TW:(k + 1) * TW, :])
        s0 += prods[3 * TW, :].sum()
    num = W_BASE * s0 + DW1 * s_band[0] + DW2 * s_band[1] + DW3 * s_band[2]
    den = W_BASE * N_TOTAL + DW1 * c[0] + DW2 * c[1] + DW3 * c[2]
    return np.array(num / den, dtype=np.float32)


def kernel(y_pred: np.ndarray, y_true: np.ndarray) -> np.ndarray:
    try:
        res = _run_device(y_pred, y_true)
    except Exception:
        import time as _time
        _time.sleep(2.0)
        res = _run_device(y_pred, y_true)
    return _finalize(res.results)


# revision 53
# speedup vs baseline: 1.0447x; 1.0096x over previous
"""Weighted-MAE loss (nn_MAELoss) on 8 Trainium2 NeuronCores.

reference:  w = bucket-weights(y_true) via thresholds log1p(5/25/50),
            loss = sum(w * |y_true - y_pred|) / sum(w)

Strategy: data-parallel over the batch dim (8 shards of 8 batches). Inputs
are cast to bf16 on the host (rel err ~2.7e-4, far inside the 2e-2 gate),
halving HBM traffic: 7.86 MB/core -> ~21.8us DMA floor at 360 GB/s.

Per-core dataflow (all stock ops; every engine stays under the DMA floor):
  DMA   : yt/yp stream into full resident SBUF buffers in column chunks
          on the SP queue (which carries nothing else, so it never stalls).
  DVE   : tensor_scalar is_ge builds the three bf16 threshold masks into a
          packed [m1|m2|m3|ones] stationary layout (4x perf mode, bucket
          counts fall out of accum_out); most spans' d = yt - yp run here
          too (2x perf mode).
  Pool  : a few mid-stream spans take BOTH their sub and abs on GPSIMD
          ("vertical" ownership) so a slow Pool op never head-blocks the
          in-order DVE/ACT queues.
  ACT   : absd = Abs(d) for the DVE-owned spans.
  PE    : per 40-col microtile, matmul with stationary = [m1|m2|m3|ones]
          (121 cols) and moving = absd (40 cols), all accumulated into one
          PSUM bank. psum[40k+i, j] += sum_p m_k[p,i]*absd[p,j] and row 120
          accumulates plain column sums of absd. The host reads the three
          40-wide diagonal bands (S_k = sum(m_k * |d|)) and row 120 (S0) --
          no elementwise product pass and no reduction pass ever run.
The host combines counts and S0..S3 in float64 and divides.
"""

import os
import sys

import numpy as np

try:
    import concourse  # noqa: F401
except ImportError:  # pragma: no cover
    for _p in ("/root/.axon_site/_ro/trn_rl_repo", "/opt/trn_rl_repo"):
        if os.path.isdir(_p) and _p not in sys.path:
            sys.path.append(_p)

from contextlib import ExitStack

import concourse.bacc as bacc
import concourse.tile as tile
from concourse import mybir
from concourse.bass_utils import run_bass_kernel_spmd

# ----------------------------------------------------------------- problem
N_CORES = 8
B, C, T, H, W = 64, 1, 15, 128, 128
SHARD_B = B // N_CORES
P = 128
F = SHARD_B * C * T * H * W // P  # 15360
N_TOTAL = B * C * T * H * W      # 15728640

TW = 40                   # microtile width (3*TW + 1 = 121 <= 128 stationary)
NT = F // TW              # 384 microtiles
SW = 3 * TW + 1           # stationary width incl. ones column
NP = SW                   # psum partition rows
assert NT * TW == F

THR1 = float(np.float32(np.log1p(5.0)))
THR2 = float(np.float32(np.log1p(25.0)))
THR3 = float(np.float32(np.log1p(50.0)))
W_BASE = 0.2
DW1, DW2, DW3 = 29.8, 2470.0, 17500.0

# DMA chunks (tiles): a chunk pair needs >= ~22 tiles for its transfer
# time to cover the 2x625ns HWDGE fixed cost, so only the first chunk
# (compute warm-up) and the last (drain) are small
CHUNKS_T = [8, 24, 24, 32, 40, 48, 48, 48, 48, 32, 24, 8]
assert sum(CHUNKS_T) == NT
# mask-op groups == chunks (fine-grained waits at every boundary)
MGROUPS = [(i,) for i in range(len(CHUNKS_T))]
NG = len(MGROUPS)
# sub/abs work spans (tiles): chunk-aligned splits of <= 20 tiles
SPANS_T = [8, 12, 12, 12, 12, 16, 16, 20, 20] + [16] * 14 + [12, 12, 8]
assert sum(SPANS_T) == NT
NS = len(SPANS_T)
# spans whose sub runs on GPSIMD (alternating mid spans, none near the
# tail); their abs and matmuls are emitted LAG spans late so slow GPSIMD
# work never head-blocks the in-order ACT/PE queues
POOL_SPANS = {5, 7, 9, 11, 13, 15, 17, 19}
LAG = 1
# tail spans where |d| comes from a DVE relu pair (relu(d), relu(-d))
# with doubled matmuls, keeping ACT's backlog off the drain chain
RELU_SPANS = {NS - 2, NS - 1}

ND = 3 * NG           # acc slots: (c1, c2, c3) per mask group
NOUT = ND + TW        # + staged psum cols

_STATE: dict = {}


def _build():
    if "nc" in _STATE:
        return _STATE["nc"]
    f32 = mybir.dt.float32
    bf16 = mybir.dt.bfloat16
    A = mybir.AluOpType
    nc = bacc.Bacc("TRN2", target_bir_lowering=False, debug=False,
                   enable_asserts=False)
    yt_d = nc.dram_tensor("y_true", [P, NT, TW], bf16, kind="ExternalInput").ap()
    yp_d = nc.dram_tensor("y_pred", [P, NT, TW], bf16, kind="ExternalInput").ap()
    acc_d = nc.dram_tensor("partials", [P, NOUT], f32, kind="ExternalOutput").ap()

    chunk_ends = np.cumsum(CHUNKS_T).tolist()
    chunk_starts = [0] + chunk_ends[:-1]
    span_ends = np.cumsum(SPANS_T).tolist()
    span_starts = [0] + span_ends[:-1]

    with tile.TileContext(nc) as tc, ExitStack() as ctx:
        big_pool = ctx.enter_context(tc.tile_pool(name="big", bufs=1))
        acc_pool = ctx.enter_context(tc.tile_pool(name="acc", bufs=1))
        ps_pool = ctx.enter_context(tc.psum_pool(name="ps", bufs=1))

        yt = big_pool.tile([P, NT, TW], bf16, tag="yt")
        yp = big_pool.tile([P, NT, TW], bf16, tag="yp")
        masks = big_pool.tile([P, NT, SW], bf16, tag="masks")
        # d holds yt-yp per span, then |d| (or relu(d)) after the in-place
        # second pass (a single full-size tensor: no rotating-buffer WAR
        # stalls, minimal SBUF)
        dfull = big_pool.tile([P, NT, TW], bf16, tag="d")
        # relu(-d) halves for the RELU_SPANS tail tiles
        rneg = big_pool.tile([P, 24, TW], bf16, tag="rneg")
        acc = acc_pool.tile([P, NOUT], f32, tag="acc")
        psum = ps_pool.tile([NP, TW], f32, tag="ps")

        # ones column of the stationary (psum row 120 = column sums of absd);
        # on Pool, which is otherwise idle until mid-stream
        nc.gpsimd.memset(masks[:, :, 3 * TW:SW], 1.0)

        # --- DMA order: the tail chunks' yt is pulled EARLY so their mask
        # work is done long before the stream ends; the last-landing data
        # (tail yp) then needs only sub+relu+matmul in the drain ------------
        NCH = len(CHUNKS_T)
        # prefetch ONLY the last chunk's yt (its masks run at ~4us, so the
        # last-landing yp needs just sub+relu+matmul in the drain)
        dma_order = [("yt", 0), ("yp", 0), ("yt", NCH - 1)]
        for ci in range(1, NCH - 1):
            dma_order.append(("yt", ci))
            dma_order.append(("yp", ci))
        dma_order.append(("yp", NCH - 1))
        spans_of_chunk = {}
        for si in range(NS):
            ci = next(i for i in range(NCH)
                      if chunk_starts[i] <= span_starts[si] < chunk_ends[i])
            assert span_ends[si] <= chunk_ends[ci], "span straddles chunk"
            spans_of_chunk.setdefault(ci, []).append(si)
        events = []
        for kind, ci in dma_order:
            events.append((kind, ci))

        pend_abs = []      # Pool spans whose abs emission is lagged
        abs_done = set()
        masks_done = set()
        mm_queue = []      # spans pending PE emission
        span_chunk = {si: ci for ci, sis in spans_of_chunk.items()
                      for si in sis}

        relu_off = {}
        for _off, _si in enumerate(sorted(RELU_SPANS)):
            relu_off[_si] = sum(SPANS_T[s] for s in sorted(RELU_SPANS)[:_off])

        def emit_abs(si):
            s0, s1 = span_starts[si], span_ends[si]
            if si in RELU_SPANS:
                # relu(-d) into scratch first (reads d), then relu(d)
                # in-place -- all on DVE at 4x, no ACT hop in the drain
                r0 = relu_off[si]
                r1 = r0 + (s1 - s0)
                nc.vector.tensor_scalar(rneg[:, r0:r1, :], dfull[:, s0:s1, :],
                                        0.0, -1.0, A.min, A.mult)
                nc.vector.tensor_scalar(dfull[:, s0:s1, :], dfull[:, s0:s1, :],
                                        0.0, 1.0, A.max, A.mult)
            else:
                nc.scalar.activation(dfull[:, s0:s1, :], dfull[:, s0:s1, :],
                                     mybir.ActivationFunctionType.Abs)
            abs_done.add(si)

        def emit_matmuls(si):
            # psum accumulation is order-independent; only the start
            # (span 0, tile 0, emitted first) and stop (span NS-1, tile
            # NT-1, emitted last) flags are order-sensitive
            s0, s1 = span_starts[si], span_ends[si]
            for tt in range(s0, s1):
                last = tt == NT - 1
                nc.tensor.matmul(
                    psum[:, :],
                    masks[:, tt, :],          # [P, SW] stationary
                    dfull[:, tt, :],          # [P, TW] moving (|d| or relu)
                    start=tt == 0, stop=last and si not in RELU_SPANS)
                if si in RELU_SPANS:
                    rt = relu_off[si] + (tt - s0)
                    nc.tensor.matmul(
                        psum[:, :],
                        masks[:, tt, :],
                        rneg[:, rt, :],       # relu(-d) half
                        start=False, stop=last)

        def drain_mm(final=False):
            # emit any span whose masks and |d| already exist (Tile
            # discovers deps in emission order, and the in-order PE queue
            # would head-block on a not-yet-ready span's matmuls); span 0
            # must go first and span NS-1 last
            for qsi in sorted(mm_queue):
                if span_chunk[qsi] not in masks_done or qsi not in abs_done:
                    continue
                if qsi == NS - 1 and not (final and len(mm_queue) == 1):
                    continue
                if 0 in mm_queue and qsi != 0:
                    continue
                mm_queue.remove(qsi)
                emit_matmuls(qsi)

        def emit_masks(g0, g1, slot):
            for k, thr in enumerate((THR1, THR2, THR3)):
                # with accum_out, op1 is the reduction op: accum=sum(mask)
                nc.vector.tensor_scalar(
                    masks[:, g0:g1, k * TW:(k + 1) * TW],
                    yt[:, g0:g1, :], thr, 0.0,
                    A.is_ge, A.add,
                    accum_out=acc[:, 3 * slot + k:3 * slot + k + 1])

        for kind, ci in events:
            c0, c1 = chunk_starts[ci], chunk_ends[ci]
            if kind == "yt":
                nc.sync.dma_start(yt[:, c0:c1, :], yt_d[:, c0:c1, :])
                emit_masks(c0, c1, ci)
                masks_done.add(ci)
                drain_mm()
                continue
            nc.sync.dma_start(yp[:, c0:c1, :], yp_d[:, c0:c1, :])
            for si in spans_of_chunk[ci]:
                s0, s1 = span_starts[si], span_ends[si]
                yt_s, yp_s = yt[:, s0:s1, :], yp[:, s0:s1, :]
                d_s = dfull[:, s0:s1, :]
                on_pool = si in POOL_SPANS
                if on_pool:
                    nc.gpsimd.tensor_sub(d_s, yt_s, yp_s)
                else:
                    nc.vector.tensor_sub(d_s, yt_s, yp_s)
                mm_queue.append(si)
                if on_pool:
                    pend_abs.append(si)
                else:
                    emit_abs(si)
                while pend_abs and si - pend_abs[0] >= LAG:
                    emit_abs(pend_abs.pop(0))
                drain_mm()
        for qsi in pend_abs:
            emit_abs(qsi)
        drain_mm(final=True)
        assert not mm_queue

        # --- drain: stage psum into acc's tail region, one output DMA ------
        nc.vector.tensor_copy(acc[0:NP, ND:NOUT], psum[:, :])
        nc.sync.dma_start(acc_d[:], acc[:])

    nc.compile()
    _STATE["nc"] = nc
    return nc


def _run_device(y_pred: np.ndarray, y_true: np.ndarray, **kw):
    import ml_dtypes
    nc = _build()
    y_pred = np.asarray(y_pred, dtype=np.float32).reshape(B, -1)
    y_true = np.asarray(y_true, dtype=np.float32).reshape(B, -1)
    in_maps = []
    for c in range(N_CORES):
        sl = slice(c * SHARD_B, (c + 1) * SHARD_B)
        in_maps.append({
            "y_true": np.ascontiguousarray(y_true[sl]).astype(
                ml_dtypes.bfloat16).reshape(P, NT, TW),
            "y_pred": np.ascontiguousarray(y_pred[sl]).astype(
                ml_dtypes.bfloat16).reshape(P, NT, TW),
        })
    return run_bass_kernel_spmd(nc, in_maps, list(range(N_CORES)), **kw)


def _finalize(results) -> np.ndarray:
    c = np.zeros(3, dtype=np.float64)
    s_band = np.zeros(3, dtype=np.float64)
    s0 = 0.0
    for ci in range(N_CORES):
        part = results[ci]["partials"].astype(np.float64)
        sl = part[:, :ND].sum(axis=0).reshape(NG, 3)
        c += sl.sum(axis=0)
        prods = part[0:NP, ND:NOUT]  # staged psum [121, TW]
        for k in range(3):
            s_band[k] += np.trace(prods[k * TW:(k + 1) * TW, :])
        s0 += prods[3 * TW, :].sum()
    num = W_BASE * s0 + DW1 * s_band[0] + DW2 * s_band[1] + DW3 * s_band[2]
    den = W_BASE * N_TOTAL + DW1 * c[0] + DW2 * c[1] + DW3 * c[2]
    return np.array(num / den, dtype=np.float32)


def kernel(y_pred: np.ndarray, y_true: np.ndarray) -> np.ndarray:
    try:
        res = _run_device(y_pred, y_true)
    except Exception:
        import time as _time
        _time.sleep(2.0)
        res = _run_device(y_pred, y_true)
    return _finalize(res.results)


# revision 54
# speedup vs baseline: 1.0657x; 1.0201x over previous
"""Weighted-MAE loss (nn_MAELoss) on 8 Trainium2 NeuronCores.

reference:  w = bucket-weights(y_true) via thresholds log1p(5/25/50),
            loss = sum(w * |y_true - y_pred|) / sum(w)

Strategy: data-parallel over the batch dim (8 shards of 8 batches). Inputs
are cast to bf16 on the host (rel err ~2.7e-4, far inside the 2e-2 gate),
halving HBM traffic: 7.86 MB/core -> ~21.8us DMA floor at 360 GB/s.

Per-core dataflow (all stock ops; every engine stays under the DMA floor):
  DMA   : yt/yp stream into full resident SBUF buffers in column chunks
          on the SP queue (which carries nothing else, so it never stalls).
  DVE   : tensor_scalar is_ge builds the three bf16 threshold masks into a
          packed [m1|m2|m3|ones] stationary layout (4x perf mode, bucket
          counts fall out of accum_out); most spans' d = yt - yp run here
          too (2x perf mode).
  Pool  : a few mid-stream spans take BOTH their sub and abs on GPSIMD
          ("vertical" ownership) so a slow Pool op never head-blocks the
          in-order DVE/ACT queues.
  ACT   : absd = Abs(d) for the DVE-owned spans.
  PE    : per 40-col microtile, matmul with stationary = [m1|m2|m3|ones]
          (121 cols) and moving = absd (40 cols), all accumulated into one
          PSUM bank. psum[40k+i, j] += sum_p m_k[p,i]*absd[p,j] and row 120
          accumulates plain column sums of absd. The host reads the three
          40-wide diagonal bands (S_k = sum(m_k * |d|)) and row 120 (S0) --
          no elementwise product pass and no reduction pass ever run.
The host combines counts and S0..S3 in float64 and divides.
"""

import os
import sys

import numpy as np

try:
    import concourse  # noqa: F401
except ImportError:  # pragma: no cover
    for _p in ("/root/.axon_site/_ro/trn_rl_repo", "/opt/trn_rl_repo"):
        if os.path.isdir(_p) and _p not in sys.path:
            sys.path.append(_p)

from contextlib import ExitStack

import concourse.bacc as bacc
import concourse.tile as tile
from concourse import mybir
from concourse.bass_utils import run_bass_kernel_spmd

# ----------------------------------------------------------------- problem
N_CORES = 8
B, C, T, H, W = 64, 1, 15, 128, 128
SHARD_B = B // N_CORES
P = 128
F = SHARD_B * C * T * H * W // P  # 15360
N_TOTAL = B * C * T * H * W      # 15728640

TW = 40                   # microtile width (3*TW + 1 = 121 <= 128 stationary)
NT = F // TW              # 384 microtiles
SW = 3 * TW + 1           # stationary width incl. ones column
NP = SW                   # psum partition rows
assert NT * TW == F

THR1 = float(np.float32(np.log1p(5.0)))
THR2 = float(np.float32(np.log1p(25.0)))
THR3 = float(np.float32(np.log1p(50.0)))
W_BASE = 0.2
DW1, DW2, DW3 = 29.8, 2470.0, 17500.0

# DMA chunks (tiles): a chunk pair needs >= ~22 tiles for its transfer
# time to cover the 2x625ns HWDGE fixed cost, so only the first chunk
# (compute warm-up) and the last (drain) are small
CHUNKS_T = [8, 24, 24, 32, 40, 48, 48, 48, 48, 32, 24, 8]
assert sum(CHUNKS_T) == NT
# mask-op groups == chunks (fine-grained waits at every boundary)
MGROUPS = [(i,) for i in range(len(CHUNKS_T))]
NG = len(MGROUPS)
# sub/abs work spans (tiles): chunk-aligned splits of <= 20 tiles
SPANS_T = [8, 12, 12, 12, 12, 16, 16, 20, 20] + [16] * 14 + [12, 12, 8]
assert sum(SPANS_T) == NT
NS = len(SPANS_T)
# spans whose sub runs on GPSIMD (alternating mid spans, none near the
# tail); their abs and matmuls are emitted LAG spans late so slow GPSIMD
# work never head-blocks the in-order ACT/PE queues
POOL_SPANS = {5, 7, 9, 11, 13, 15, 17, 19}
LAG = 1
# tail spans where |d| comes from a DVE relu pair (relu(d), relu(-d))
# with doubled matmuls, keeping ACT's backlog off the drain chain
RELU_SPANS = {NS - 2, NS - 1}

ND = 3 * NG           # acc slots: (c1, c2, c3) per mask group
NOUT = ND + TW        # + staged psum cols

_STATE: dict = {}


def _build():
    if "nc" in _STATE:
        return _STATE["nc"]
    f32 = mybir.dt.float32
    bf16 = mybir.dt.bfloat16
    A = mybir.AluOpType
    nc = bacc.Bacc("TRN2", target_bir_lowering=False, debug=False,
                   enable_asserts=False)
    yt_d = nc.dram_tensor("y_true", [P, NT, TW], bf16, kind="ExternalInput").ap()
    yp_d = nc.dram_tensor("y_pred", [P, NT, TW], bf16, kind="ExternalInput").ap()
    acc_d = nc.dram_tensor("partials", [P, NOUT], f32, kind="ExternalOutput").ap()

    chunk_ends = np.cumsum(CHUNKS_T).tolist()
    chunk_starts = [0] + chunk_ends[:-1]
    span_ends = np.cumsum(SPANS_T).tolist()
    span_starts = [0] + span_ends[:-1]

    with tile.TileContext(nc) as tc, ExitStack() as ctx:
        big_pool = ctx.enter_context(tc.tile_pool(name="big", bufs=1))
        acc_pool = ctx.enter_context(tc.tile_pool(name="acc", bufs=1))
        ps_pool = ctx.enter_context(tc.psum_pool(name="ps", bufs=1))

        yt = big_pool.tile([P, NT, TW], bf16, tag="yt")
        yp = big_pool.tile([P, NT, TW], bf16, tag="yp")
        masks = big_pool.tile([P, NT, SW], bf16, tag="masks")
        # d holds yt-yp per span, then |d| (or relu(d)) after the in-place
        # second pass (a single full-size tensor: no rotating-buffer WAR
        # stalls, minimal SBUF)
        dfull = big_pool.tile([P, NT, TW], bf16, tag="d")
        # relu(-d) halves for the RELU_SPANS tail tiles
        rneg = big_pool.tile([P, 24, TW], bf16, tag="rneg")
        acc = acc_pool.tile([P, NOUT], f32, tag="acc")
        psum = ps_pool.tile([NP, TW], f32, tag="ps")

        # ones column of the stationary (psum row 120 = column sums of absd);
        # on Pool, which is otherwise idle until mid-stream
        nc.gpsimd.memset(masks[:, :, 3 * TW:SW], 1.0)

        # --- DMA order: the tail chunks' yt is pulled EARLY so their mask
        # work is done long before the stream ends; the last-landing data
        # (tail yp) then needs only sub+relu+matmul in the drain ------------
        NCH = len(CHUNKS_T)
        dma_order = []
        for ci in range(NCH):
            dma_order.append(("yt", ci))
            dma_order.append(("yp", ci))
        spans_of_chunk = {}
        for si in range(NS):
            ci = next(i for i in range(NCH)
                      if chunk_starts[i] <= span_starts[si] < chunk_ends[i])
            assert span_ends[si] <= chunk_ends[ci], "span straddles chunk"
            spans_of_chunk.setdefault(ci, []).append(si)
        events = []
        for kind, ci in dma_order:
            events.append((kind, ci))

        pend_abs = []      # Pool spans whose abs emission is lagged
        abs_done = set()
        masks_done = set()
        mm_queue = []      # spans pending PE emission
        span_chunk = {si: ci for ci, sis in spans_of_chunk.items()
                      for si in sis}

        relu_off = {}
        for _off, _si in enumerate(sorted(RELU_SPANS)):
            relu_off[_si] = sum(SPANS_T[s] for s in sorted(RELU_SPANS)[:_off])

        def emit_abs(si):
            s0, s1 = span_starts[si], span_ends[si]
            if si in RELU_SPANS:
                # relu(-d) into scratch first (reads d), then relu(d)
                # in-place -- all on DVE at 4x, no ACT hop in the drain
                r0 = relu_off[si]
                r1 = r0 + (s1 - s0)
                nc.vector.tensor_scalar(rneg[:, r0:r1, :], dfull[:, s0:s1, :],
                                        0.0, -1.0, A.min, A.mult)
                nc.vector.tensor_scalar(dfull[:, s0:s1, :], dfull[:, s0:s1, :],
                                        0.0, 1.0, A.max, A.mult)
            else:
                nc.scalar.activation(dfull[:, s0:s1, :], dfull[:, s0:s1, :],
                                     mybir.ActivationFunctionType.Abs)
            abs_done.add(si)

        def emit_matmuls(si):
            # psum accumulation is order-independent; only the start
            # (span 0, tile 0, emitted first) and stop (span NS-1, tile
            # NT-1, emitted last) flags are order-sensitive
            s0, s1 = span_starts[si], span_ends[si]
            for tt in range(s0, s1):
                last = tt == NT - 1
                nc.tensor.matmul(
                    psum[:, :],
                    masks[:, tt, :],          # [P, SW] stationary
                    dfull[:, tt, :],          # [P, TW] moving (|d| or relu)
                    start=tt == 0, stop=last and si not in RELU_SPANS)
                if si in RELU_SPANS:
                    rt = relu_off[si] + (tt - s0)
                    nc.tensor.matmul(
                        psum[:, :],
                        masks[:, tt, :],
                        rneg[:, rt, :],       # relu(-d) half
                        start=False, stop=last)

        def drain_mm(final=False):
            # emit any span whose masks and |d| already exist (Tile
            # discovers deps in emission order, and the in-order PE queue
            # would head-block on a not-yet-ready span's matmuls); span 0
            # must go first and span NS-1 last
            for qsi in sorted(mm_queue):
                if span_chunk[qsi] not in masks_done or qsi not in abs_done:
                    continue
                if qsi == NS - 1 and not (final and len(mm_queue) == 1):
                    continue
                if 0 in mm_queue and qsi != 0:
                    continue
                mm_queue.remove(qsi)
                emit_matmuls(qsi)

        def emit_masks(g0, g1, slot):
            for k, thr in enumerate((THR1, THR2, THR3)):
                # with accum_out, op1 is the reduction op: accum=sum(mask)
                nc.vector.tensor_scalar(
                    masks[:, g0:g1, k * TW:(k + 1) * TW],
                    yt[:, g0:g1, :], thr, 0.0,
                    A.is_ge, A.add,
                    accum_out=acc[:, 3 * slot + k:3 * slot + k + 1])

        for kind, ci in events:
            c0, c1 = chunk_starts[ci], chunk_ends[ci]
            if kind == "yt":
                nc.sync.dma_start(yt[:, c0:c1, :], yt_d[:, c0:c1, :])
                emit_masks(c0, c1, ci)
                masks_done.add(ci)
                drain_mm()
                continue
            nc.sync.dma_start(yp[:, c0:c1, :], yp_d[:, c0:c1, :])
            for si in spans_of_chunk[ci]:
                s0, s1 = span_starts[si], span_ends[si]
                yt_s, yp_s = yt[:, s0:s1, :], yp[:, s0:s1, :]
                d_s = dfull[:, s0:s1, :]
                on_pool = si in POOL_SPANS
                if on_pool:
                    nc.gpsimd.tensor_sub(d_s, yt_s, yp_s)
                else:
                    nc.vector.tensor_sub(d_s, yt_s, yp_s)
                mm_queue.append(si)
                if on_pool:
                    pend_abs.append(si)
                else:
                    emit_abs(si)
                while pend_abs and si - pend_abs[0] >= LAG:
                    emit_abs(pend_abs.pop(0))
                drain_mm()
        for qsi in pend_abs:
            emit_abs(qsi)
        drain_mm(final=True)
        assert not mm_queue

        # --- drain: stage psum into acc's tail region, one output DMA ------
        nc.vector.tensor_copy(acc[0:NP, ND:NOUT], psum[:, :])
        nc.sync.dma_start(acc_d[:], acc[:])

    nc.compile()
    _STATE["nc"] = nc
    return nc


def _run_device(y_pred: np.ndarray, y_true: np.ndarray, **kw):
    import ml_dtypes
    nc = _build()
    y_pred = np.asarray(y_pred, dtype=np.float32).reshape(B, -1)
    y_true = np.asarray(y_true, dtype=np.float32).reshape(B, -1)
    in_maps = []
    for c in range(N_CORES):
        sl = slice(c * SHARD_B, (c + 1) * SHARD_B)
        in_maps.append({
            "y_true": np.ascontiguousarray(y_true[sl]).astype(
                ml_dtypes.bfloat16).reshape(P, NT, TW),
            "y_pred": np.ascontiguousarray(y_pred[sl]).astype(
                ml_dtypes.bfloat16).reshape(P, NT, TW),
        })
    return run_bass_kernel_spmd(nc, in_maps, list(range(N_CORES)), **kw)


def _finalize(results) -> np.ndarray:
    c = np.zeros(3, dtype=np.float64)
    s_band = np.zeros(3, dtype=np.float64)
    s0 = 0.0
    for ci in range(N_CORES):
        part = results[ci]["partials"].astype(np.float64)
        sl = part[:, :ND].sum(axis=0).reshape(NG, 3)
        c += sl.sum(axis=0)
        prods = part[0:NP, ND:NOUT]  # staged psum [121, TW]
        for k in range(3):
            s_band[k] += np.trace(prods[k * TW:(k + 1) * TW, :])
        s0 += prods[3 * TW, :].sum()
    num = W_BASE * s0 + DW1 * s_band[0] + DW2 * s_band[1] + DW3 * s_band[2]
    den = W_BASE * N_TOTAL + DW1 * c[0] + DW2 * c[1] + DW3 * c[2]
    return np.array(num / den, dtype=np.float32)


def kernel(y_pred: np.ndarray, y_true: np.ndarray) -> np.ndarray:
    try:
        res = _run_device(y_pred, y_true)
    except Exception:
        import time as _time
        _time.sleep(2.0)
        res = _run_device(y_pred, y_true)
    return _finalize(res.results)
